# revision 45
# baseline (speedup 1.0000x reference)
"""Trainium2 Bass kernel for nn_Loss_PIP (PIP loss: box region terms + distance-map
weighted cross-entropy).

Strategy (data-parallel over batch across 8 NeuronCores, 2 images/core):
  - Pixel layout: the host deals each image's 65536 pixels into a
    [128 partitions x 512 column] half of the tile SORTED BY LABEL,
    round-robin across partitions (slot k -> partition k%128, column k//128;
    image b occupies columns [512b, 512b+512)). Every per-pixel computation
    (exp, denominator accumulation, den export, gather) is permutation-
    invariant, but the label-gather now only scans the narrow column range
    where class c of image b lives (~30 columns instead of 1024): the DVE
    gather drops from ~23.7us to ~4us and stops being the bottleneck. The
    class column ranges are computed from the actual labels at first call
    and compiled into the program (max across cores; stray neighbor-class
    pixels inside a range are killed by the enc gate).
  - Device (per core, SPMD): stream the 21 logit channel planes in fp16 (half
    the HBM bytes of f32); ACT computes exp (bf16, dual-channel ops to
    amortize fixed cost); PE accumulates the softmax denominator in PSUM via
    identity-matmul (identity built on-device on Pool); DVE runs one fused
    PIP_GATHER_DOT pass per (image, class) column range (driven by
    enc = 2*label + w) and the PSUM->SBUF den copies. The denominator is
    exported raw fp16; the host takes the log (no Ln => no activation-table
    switch). The last channel runs as two half planes to shorten the closing
    exp->matmul->copy->DMA chain; PSUM/staging tiles are split per bank to
    avoid false tile-granular WARs.
  - Host: Gamma weight-map pipeline (bbox-only), per-box window reductions on
    logden/logits, the w*logden reduction, permutation (un)packing, assembly.
"""

import sys

sys.path.insert(0, "/opt/trn_rl_repo")

import numpy as np

B, C, H, W = 16, 21, 256, 256
NB = 20
N_CORES = 8
IPC = B // N_CORES  # images per core
LAMB, ALPHA, TAU, R, SIGMA = 1.0, 0.5, 1.0, 3, 1.0
IGNORE = 255
F = 4 * W  # 1024 free elems per partition
HB = F // 2  # psum bank width in f32 = per-image column half
IPIX = H * W  # pixels per image = 128 * HB

_CACHE = {}


def _register_fused_op():
    """Register PIP_GATHER_DOT: out = m*(enc-s0)*in1, m = (enc-s0) in (s1, imm2);
    accum_out = sum(out). With enc = 2*label + w (w in {0} U (1,1.5)), s0=2c,
    s1=0.5, imm2=1.5 this computes w*(label==c)*logit in one DVE pass."""
    from concourse import dve_ops
    from concourse.dve_spec import C0, C1, C2, Spec, Src0, Src1, Zero, lower
    from concourse.dve_spec import _has_src1 as has_src1
    from concourse.dve_uop import DveOpSpec
    from operator import add as op_add
    import numpy as np_

    name = "PIP_GATHER_DOT"
    if name in dve_ops._SUB_OPCODE_FOR_NAME:
        return next(o for o in dve_ops.OPS if o.name == name)

    _t = Src0 - C0

    def _ref(in0, in1, s0, s1, imm2):
        t = in0.astype(np_.float32) - s0
        m = ((t > s1) & (t < imm2)).astype(np_.float32)
        b = (m * t * in1).astype(np_.float32)
        return b, b.reshape(b.shape[0], -1).sum(axis=-1, keepdims=True)

    spec = Spec(
        body=((_t > C1) & (_t < C2)) * _t * Src1,
        accum=op_add,
        accum_init=Zero,
        reference=_ref,
    )
    row = dve_ops._CUSTOM_DVE_ROW_BASE + len(dve_ops.OPS)
    assert row < 0x20
    shas = {}
    for ver in ("v3", "v4"):
        try:
            uops = lower(spec, ver=ver)
        except Exception:
            continue
        shas[ver] = DveOpSpec(
            name=name, opcode=row, uops=uops, rd1_en=has_src1(spec)
        ).sha(ver)
    op = dve_ops.DveOp(name, spec, subdim=False, uops_sha=shas)
    dve_ops.OPS.append(op)
    dve_ops.CUSTOM_DVE_SPECS[name] = spec
    dve_ops._SUB_OPCODE_FOR_NAME[name] = row
    return op


def _build_nc(ranges):
    """ranges[b][c] = (start, end) absolute column range (image b's half)
    covering every partition's class-c pixels across all cores."""
    import concourse.bacc as bacc
    import concourse.mybir as mybir
    from concourse import tile

    dt = mybir.dt
    Alu = mybir.AluOpType
    Act = mybir.ActivationFunctionType

    nc = bacc.Bacc(
        "TRN2",
        target_bir_lowering=False,
        debug=False,
        enable_asserts=False,
        num_devices=N_CORES,
    )

    # host supplies logits label-sorted + fp16: [c, p, f] = logits[pix(p,f)]
    logits16 = nc.dram_tensor("logits16", [C, 128, F], dt.float16, kind="ExternalInput")
    enc_in = nc.dram_tensor("enc", [128, F], dt.float16, kind="ExternalInput")
    den_out = nc.dram_tensor("den", [128, F], dt.float16, kind="ExternalOutput")
    # parts col 2c+b = sum_p w*(label==c)*logit_c over image b
    partials_out = nc.dram_tensor(
        "partials", [128, 2 * C], dt.float32, kind="ExternalOutput"
    )

    fused = _register_fused_op()

    with tile.TileContext(nc) as tc:
        with (
            tc.tile_pool(name="persist", bufs=1) as pp,
            tc.tile_pool(name="stream", bufs=4) as sp,
            tc.tile_pool(name="psum", bufs=1, space="PSUM") as psp,
        ):
            enc = pp.tile([128, F], dt.float16, name="enc")
            ident = pp.tile([128, 128], dt.bfloat16, name="ident")
            ones = pp.tile([128, 128], dt.bfloat16, name="ones")
            parts = pp.tile([128, 2 * C], dt.float32, name="parts")
            # separate PSUM/SBUF tiles per bank half: no false WAR between the
            # bank-0 epilogue and bank-1 accumulation
            dps = [psp.tile([128, HB], dt.float32, name=f"dps{h}") for h in range(2)]
            dsb = [pp.tile([128, HB], dt.float16, name=f"dsb{h}") for h in range(2)]

            # identity for the PE accumulate, generated on the Pool engine
            nc.gpsimd.memset(ones[:, :], 1.0)
            nc.gpsimd.affine_select(
                out=ident[:, :],
                in_=ones[:, :],
                pattern=[[1, 128]],
                compare_op=Alu.is_equal,
                fill=0.0,
                base=0,
                channel_multiplier=-1,
            )

            # ---- input stream on the SP queue ----------------------------
            lg = {}
            lg_dual = {}

            def dma_lg(c):
                t = sp.tile([128, F], dt.float16, name=f"lg{c}", tag="lg", bufs=C)
                nc.sync.dma_start(out=t[:, :], in_=logits16[c])
                lg[c] = t

            def dma_lgn(c, n):  # channels c..c+n-1 in one DMA / one tile
                t = sp.tile(
                    [128, n * F], dt.float16, name=f"lg{c}", tag=f"lg{n}x", bufs=4
                )
                nc.sync.dma_start(
                    out=t[:, :].rearrange("p (c f) -> p c f", c=n),
                    in_=logits16[c : c + n].rearrange("c p f -> p c f"),
                )
                lg_dual[c] = t
                for k in range(n):
                    lg[c + k] = t[:, k * F : (k + 1) * F]

            # exp groups grow as the pipeline warms (amortizing ACT's 185ns
            # per-op cost), then shrink again near the tail so the PE matmuls
            # of the last groups don't bunch up and delay the stop matmuls
            GROUPS = [(0,), (1,), (2, 3), (4, 5), (6, 7, 8), (9, 10, 11),
                      (12, 13, 14, 15), (16, 17), (18, 19)]
            # enc rides late: the DVE gathers are tiny and have slack, while
            # ACT is arrival-paced at the head — the early slots go to lg
            for g in GROUPS[:7]:
                dma_lgn(g[0], len(g))
            nc.sync.dma_start(out=enc[:, :], in_=enc_in[:, :])
            for g in GROUPS[7:]:
                dma_lgn(g[0], len(g))
            # last channel in two half tiles so the tail chain is one half
            lg20 = [
                sp.tile([128, HB], dt.float16, name=f"lg20{h}", tag="lg20", bufs=2)
                for h in range(2)
            ]
            for h in range(2):
                nc.sync.dma_start(
                    out=lg20[h][:, :], in_=logits16[20][:, h * HB : (h + 1) * HB]
                )

            # ---- per-channel compute -------------------------------------
            wmax = max(e - s for rb in ranges for (s, e) in rb)

            def gather_dve(c, in1_tile, col0):
                # one pass per image half over class c's column range
                for b in range(2):
                    s, e = ranges[b][c]
                    tout = sp.tile(
                        [128, wmax], dt.float16, name="tout", tag="tout", bufs=2
                    )
                    nc.vector._custom_dve(
                        fused,
                        out=tout[:, : e - s],
                        in0=enc[:, s:e],
                        in1=in1_tile[:, s - col0 : e - col0],
                        s0=2.0 * c,
                        s1=0.5,
                        imm2=1.5,
                        accum_out=parts[:, 2 * c + b : 2 * c + b + 1],
                    )

            for gi, g in enumerate(GROUPS):
                n = len(g)
                exg = sp.tile(
                    [128, n * F], dt.bfloat16, name="ex", tag=f"ex{n}x", bufs=3
                )
                nc.scalar.activation(
                    out=exg[:, :], in_=lg_dual[g[0]][:, :], func=Act.Exp
                )
                for k, cc in enumerate(g):
                    exk = exg[:, k * F : (k + 1) * F]
                    for h in range(2):
                        nc.tensor.matmul(
                            dps[h][:, :],
                            ident[:, :],
                            exk[:, h * HB : (h + 1) * HB],
                            start=(cc == 0),
                            stop=False,
                        )
                    gather_dve(cc, lg[cc], 0)
            # c20: per-half exp -> stop matmul; image b's class-20 range sits
            # inside half-tile b (class 20 sorts just before the IGNORE block)
            assert ranges[0][20][1] <= HB and ranges[1][20][0] >= HB, ranges
            exh = []
            for h in range(2):
                e = sp.tile([128, HB], dt.bfloat16, name="exh", tag="exh", bufs=2)
                nc.scalar.activation(out=e[:, :], in_=lg20[h][:, :], func=Act.Exp)
                exh.append(e)
            for h in range(2):
                nc.tensor.matmul(
                    dps[h][:, :], ident[:, :], exh[h][:, :], start=False, stop=True
                )
            for b in range(2):
                s, e = ranges[b][20]
                tout = sp.tile(
                    [128, wmax], dt.float16, name="tout", tag="tout", bufs=2
                )
                nc.vector._custom_dve(
                    fused,
                    out=tout[:, : e - s],
                    in0=enc[:, s:e],
                    in1=lg20[b][:, s - b * HB : e - b * HB],
                    s0=2.0 * 20,
                    s1=0.5,
                    imm2=1.5,
                    accum_out=parts[:, 40 + b : 41 + b],
                )
            # partials out on SP before the den DMAs (it is ready earlier)
            nc.sync.dma_start(out=partials_out[:, :], in_=parts[:, :])
            # den copies run in parallel: bank 0 (ready first, after mm_a) on
            # the idle DVE with its DMA on SP; bank 1 (the critical last one)
            # on ACT right after its final exp, DMA on the same ACT queue so
            # no cross-engine sem is paid
            nc.vector.tensor_copy(out=dsb[0][:, :], in_=dps[0][:, :])
            nc.sync.dma_start(out=den_out[:, 0:HB], in_=dsb[0][:, :])
            nc.scalar.activation(out=dsb[1][:, :], in_=dps[1][:, :], func=Act.Copy)
            nc.scalar.dma_start(out=den_out[:, HB:F], in_=dsb[1][:, :])

    nc.compile()
    return nc


def _get_nc(ranges=None):
    if ranges is None:
        # introspection path (test harness): return the program built by the
        # latest kernel() call
        return next(reversed(_CACHE.values()))
    key = tuple(tuple(r) for r in (ranges[0] + ranges[1]))
    if key not in _CACHE:
        _CACHE[key] = _build_nc(ranges)
    return _CACHE[key]


def _gauss_1d():
    x = np.arange(2 * R + 1, dtype=np.float64) - R
    g = np.exp(-(x**2) / (2.0 * SIGMA**2))
    return (g / g.sum()).astype(np.float32)


def _host_gamma(bboxes):
    """Gamma weight maps [B,H,W] plus per-image Gamma sums; depends only on bboxes."""
    bb = bboxes.reshape(B * NB, 5).astype(np.int64)
    x0, y0, x1, y1, cls = bb[:, 0], bb[:, 1], bb[:, 2], bb[:, 3], bb[:, 4]
    valid = cls != -1
    ys = np.arange(H)
    xs = np.arange(W)
    row_m = (ys[None, :] >= y0[:, None]) & (ys[None, :] <= y1[:, None])  # [M,H]
    col_m = (xs[None, :] >= x0[:, None]) & (xs[None, :] <= x1[:, None])  # [M,W]
    in_r = (ys[None, :] > y0[:, None]) & (ys[None, :] < y1[:, None])
    in_c = (xs[None, :] > x0[:, None]) & (xs[None, :] < x1[:, None])

    nop = np.ones((B, H, W), dtype=np.float32)
    dis = np.zeros((B, H, W), dtype=np.float32)
    for m in range(B * NB):
        if not valid[m]:
            continue
        b = m // NB
        full = np.outer(row_m[m], col_m[m]).astype(np.float32)
        inner = np.outer(in_r[m], in_c[m]).astype(np.float32)
        nop[b] += full
        dis[b] += full * (1.0 - inner)

    g = _gauss_1d().astype(np.float64)
    # reflect-pad + separable 7x7 gaussian (matches conv with outer(g, g), 'VALID')
    disp = np.pad(dis, ((0, 0), (R, R), (0, 0)), mode="reflect").astype(np.float64)
    tmp = np.zeros((B, H, W), dtype=np.float64)
    for k in range(2 * R + 1):
        tmp += g[k] * disp[:, k : k + H, :]
    tmp = np.pad(tmp, ((0, 0), (0, 0), (R, R)), mode="reflect")
    blur = np.zeros((B, H, W), dtype=np.float64)
    for k in range(2 * R + 1):
        blur += g[k] * tmp[:, :, k : k + W]
    dis_b = blur.astype(np.float32) + 1.0

    nd = nop * dis_b
    ndmax = nd.max()
    sig = 1.0 / (1.0 + np.exp(-(nd / ndmax).astype(np.float64)))
    gam = ((sig - 0.5) * TAU + 1.0).astype(np.float32)
    s0 = gam.reshape(B, -1).astype(np.float64).sum(axis=1)  # per-image Gamma sums

    h = y1 - y0 + 1
    w = x1 - x0 + 1
    num_rc = 1e-5 + float(np.where(valid, h + w, 0).sum())
    return gam, s0, num_rc


def _host_box_terms(logits, bboxes, logden):
    """loss_rc from per-box window reductions on log-prob maps."""
    bb = bboxes.reshape(B * NB, 5).astype(np.int64)
    term = 0.0
    for m in range(B * NB):
        x0, y0, x1, y1, cls = bb[m]
        if cls == -1:
            continue
        b = m // NB
        lp = (
            logits[b, cls, y0 : y1 + 1, x0 : x1 + 1].astype(np.float64)
            - logden[b, y0 : y1 + 1, x0 : x1 + 1].astype(np.float64)
        )
        colmax = lp.max(axis=0)
        rowmax = lp.max(axis=1)
        colmin = lp.min(axis=0)
        rowmin = lp.min(axis=1)
        term += ALPHA * (colmax.sum() + rowmax.sum())
        term += (1.0 - ALPHA) * (
            np.log1p(-np.exp(colmin)).sum() + np.log1p(-np.exp(rowmin)).sum()
        )
    return -term


def _build_perms(labels):
    """Per-image label-sorted pixel permutations + per-(image-slot, class)
    absolute column ranges shared across cores. Within image b's column half,
    slot k (k = (col-512b)*128 + partition) holds pixel perm[k] of the
    image's flat [H*W] pixel index space."""
    perms = np.empty((B, IPIX), dtype=np.int64)
    Ks = np.empty((B, C + 2), dtype=np.int64)
    for i in range(B):
        lab = labels[i].reshape(-1)
        perms[i] = np.argsort(lab, kind="stable")
        cnt = np.bincount(np.minimum(lab, C), minlength=C + 1)
        Ks[i] = np.concatenate([[0], np.cumsum(cnt)])
    ranges = [[], []]
    for b in range(2):
        imgs = [i * IPC + b for i in range(N_CORES)]
        for c in range(C):
            s = min(int(Ks[i][c]) // 128 for i in imgs)
            e = max((int(Ks[i][c + 1]) - 1) // 128 + 1 for i in imgs)
            ranges[b].append((b * HB + s, b * HB + e))
    return perms, ranges


def _to_half(flat, perm):
    """flat [..., IPIX] pixel data -> [..., 128, HB] image-half layout."""
    a = np.take(flat, perm, axis=-1)
    a = a.reshape(*a.shape[:-1], HB, 128)
    return np.ascontiguousarray(np.swapaxes(a, -1, -2))


def _from_half(dev, perm):
    """[128, HB] image-half layout -> [IPIX] flat pixel data."""
    flat_k = np.ascontiguousarray(dev.T).reshape(-1)
    out = np.empty(IPIX, dtype=dev.dtype)
    out[perm] = flat_k
    return out


def kernel(logits, bboxes, labels):
    from concourse import bass_utils

    logits = np.asarray(logits, dtype=np.float32)
    bboxes = np.asarray(bboxes, dtype=np.int32)
    labels = np.asarray(labels, dtype=np.int32)

    gam, s0, num_rc = _host_gamma(bboxes)

    lab = labels.astype(np.float32)  # [B,H,W], IGNORE stays 255
    wmap = (labels != IGNORE).astype(np.float32) * gam
    enc = (2.0 * lab + wmap).astype(np.float16)  # [B,H,W]

    perms, ranges = _build_perms(labels)
    nc = _get_nc(ranges)

    lg16 = logits.astype(np.float16)  # [B,C,H,W]
    in_maps = []
    for i in range(N_CORES):
        img = [i * IPC, i * IPC + 1]
        # [C, 128, F]: image b's sorted pixels in columns [512b, 512b+512)
        lgi = np.concatenate(
            [
                _to_half(lg16[img[b]].reshape(C, IPIX), perms[img[b]])
                for b in range(2)
            ],
            axis=-1,
        )
        enci = np.concatenate(
            [_to_half(enc[img[b]].reshape(IPIX), perms[img[b]]) for b in range(2)],
            axis=-1,
        )
        in_maps.append({"logits16": lgi, "enc": enci})
    res = bass_utils.run_bass_kernel_spmd(nc, in_maps, core_ids=list(range(N_CORES)))

    logden = np.stack(
        [
            np.log(
                _from_half(
                    np.asarray(res.results[i // IPC]["den"])[
                        :, (i % IPC) * HB : (i % IPC + 1) * HB
                    ],
                    perms[i],
                ).astype(np.float32)
            ).reshape(H, W)
            for i in range(B)
        ],
        axis=0,
    )  # [B,H,W]
    loss_rc = _host_box_terms(logits, bboxes, logden)

    # weighted CE: sum w*logden (host, from exported map) - device gather sums
    wsum = (wmap.astype(np.float64) * logden.astype(np.float64)).reshape(B, -1).sum(axis=1)
    wce = 0.0
    for i in range(N_CORES):
        p = res.results[i]["partials"].astype(np.float64)
        for b in range(IPC):
            s1 = wsum[i * IPC + b] - p[:, b::2].sum()
            wce += s1 / s0[i * IPC + b]
    wce /= B

    out = LAMB * loss_rc / num_rc + wce
    return np.float32(out)


# revision 46
# speedup vs baseline: 1.0041x; 1.0041x over previous
"""Trainium2 Bass kernel for nn_Loss_PIP (PIP loss: box region terms + distance-map
weighted cross-entropy).

Strategy (data-parallel over batch across 8 NeuronCores, 2 images/core):
  - Pixel layout: the host deals each image's 65536 pixels into a
    [128 partitions x 512 column] half of the tile SORTED BY LABEL,
    round-robin across partitions (slot k -> partition k%128, column k//128;
    image b occupies columns [512b, 512b+512)). Every per-pixel computation
    (exp, denominator accumulation, den export, gather) is permutation-
    invariant, but the label-gather now only scans the narrow column range
    where class c of image b lives (~30 columns instead of 1024): the DVE
    gather drops from ~23.7us to ~4us and stops being the bottleneck. The
    class column ranges are computed from the actual labels at first call
    and compiled into the program (max across cores; stray neighbor-class
    pixels inside a range are killed by the enc gate).
  - Device (per core, SPMD): stream the 21 logit channel planes in fp16 (half
    the HBM bytes of f32); ACT computes exp (bf16, dual-channel ops to
    amortize fixed cost); PE accumulates the softmax denominator in PSUM via
    identity-matmul (identity built on-device on Pool); DVE runs one fused
    PIP_GATHER_DOT pass per (image, class) column range (driven by
    enc = 2*label + w) and the PSUM->SBUF den copies. The denominator is
    exported raw fp16; the host takes the log (no Ln => no activation-table
    switch). The last channel runs as two half planes to shorten the closing
    exp->matmul->copy->DMA chain; PSUM/staging tiles are split per bank to
    avoid false tile-granular WARs.
  - Host: Gamma weight-map pipeline (bbox-only), per-box window reductions on
    logden/logits, the w*logden reduction, permutation (un)packing, assembly.
"""

import sys

sys.path.insert(0, "/opt/trn_rl_repo")

import numpy as np

B, C, H, W = 16, 21, 256, 256
NB = 20
N_CORES = 8
IPC = B // N_CORES  # images per core
LAMB, ALPHA, TAU, R, SIGMA = 1.0, 0.5, 1.0, 3, 1.0
IGNORE = 255
F = 4 * W  # 1024 free elems per partition
HB = F // 2  # psum bank width in f32 = per-image column half
IPIX = H * W  # pixels per image = 128 * HB

_CACHE = {}


def _register_fused_op():
    """Register PIP_GATHER_DOT: out = m*(enc-s0)*in1, m = (enc-s0) in (s1, imm2);
    accum_out = sum(out). With enc = 2*label + w (w in {0} U (1,1.5)), s0=2c,
    s1=0.5, imm2=1.5 this computes w*(label==c)*logit in one DVE pass."""
    from concourse import dve_ops
    from concourse.dve_spec import C0, C1, C2, Spec, Src0, Src1, Zero, lower
    from concourse.dve_spec import _has_src1 as has_src1
    from concourse.dve_uop import DveOpSpec
    from operator import add as op_add
    import numpy as np_

    name = "PIP_GATHER_DOT"
    if name in dve_ops._SUB_OPCODE_FOR_NAME:
        return next(o for o in dve_ops.OPS if o.name == name)

    _t = Src0 - C0

    def _ref(in0, in1, s0, s1, imm2):
        t = in0.astype(np_.float32) - s0
        m = ((t > s1) & (t < imm2)).astype(np_.float32)
        b = (m * t * in1).astype(np_.float32)
        return b, b.reshape(b.shape[0], -1).sum(axis=-1, keepdims=True)

    spec = Spec(
        body=((_t > C1) & (_t < C2)) * _t * Src1,
        accum=op_add,
        accum_init=Zero,
        reference=_ref,
    )
    row = dve_ops._CUSTOM_DVE_ROW_BASE + len(dve_ops.OPS)
    assert row < 0x20
    shas = {}
    for ver in ("v3", "v4"):
        try:
            uops = lower(spec, ver=ver)
        except Exception:
            continue
        shas[ver] = DveOpSpec(
            name=name, opcode=row, uops=uops, rd1_en=has_src1(spec)
        ).sha(ver)
    op = dve_ops.DveOp(name, spec, subdim=False, uops_sha=shas)
    dve_ops.OPS.append(op)
    dve_ops.CUSTOM_DVE_SPECS[name] = spec
    dve_ops._SUB_OPCODE_FOR_NAME[name] = row
    return op


def _build_nc(ranges):
    """ranges[b][c] = (start, end) absolute column range (image b's half)
    covering every partition's class-c pixels across all cores."""
    import concourse.bacc as bacc
    import concourse.mybir as mybir
    from concourse import tile

    dt = mybir.dt
    Alu = mybir.AluOpType
    Act = mybir.ActivationFunctionType

    nc = bacc.Bacc(
        "TRN2",
        target_bir_lowering=False,
        debug=False,
        enable_asserts=False,
        num_devices=N_CORES,
    )

    # host supplies logits label-sorted + fp16: [c, p, f] = logits[pix(p,f)]
    logits16 = nc.dram_tensor("logits16", [C, 128, F], dt.float16, kind="ExternalInput")
    enc_in = nc.dram_tensor("enc", [128, F], dt.float16, kind="ExternalInput")
    den_out = nc.dram_tensor("den", [128, F], dt.float16, kind="ExternalOutput")
    # parts col 2c+b = sum_p w*(label==c)*logit_c over image b
    partials_out = nc.dram_tensor(
        "partials", [128, 2 * C], dt.float32, kind="ExternalOutput"
    )

    fused = _register_fused_op()

    with tile.TileContext(nc) as tc:
        with (
            tc.tile_pool(name="persist", bufs=1) as pp,
            tc.tile_pool(name="stream", bufs=4) as sp,
            tc.tile_pool(name="psum", bufs=1, space="PSUM") as psp,
        ):
            enc = pp.tile([128, F], dt.float16, name="enc")
            ident = pp.tile([128, 128], dt.bfloat16, name="ident")
            ones = pp.tile([128, 128], dt.bfloat16, name="ones")
            parts = pp.tile([128, 2 * C], dt.float32, name="parts")
            # separate PSUM/SBUF tiles per bank half: no false WAR between the
            # bank-0 epilogue and bank-1 accumulation
            dps = [psp.tile([128, HB], dt.float32, name=f"dps{h}") for h in range(2)]
            dsb = [pp.tile([128, HB], dt.float16, name=f"dsb{h}") for h in range(2)]

            # identity for the PE accumulate, generated on the Pool engine
            nc.gpsimd.memset(ones[:, :], 1.0)
            nc.gpsimd.affine_select(
                out=ident[:, :],
                in_=ones[:, :],
                pattern=[[1, 128]],
                compare_op=Alu.is_equal,
                fill=0.0,
                base=0,
                channel_multiplier=-1,
            )

            # ---- input stream on the SP queue ----------------------------
            lg = {}
            lg_dual = {}

            def dma_lg(c):
                t = sp.tile([128, F], dt.float16, name=f"lg{c}", tag="lg", bufs=C)
                nc.sync.dma_start(out=t[:, :], in_=logits16[c])
                lg[c] = t

            def dma_lgn(c, n):  # channels c..c+n-1 in one DMA / one tile
                t = sp.tile(
                    [128, n * F], dt.float16, name=f"lg{c}", tag=f"lg{n}x", bufs=4
                )
                nc.sync.dma_start(
                    out=t[:, :].rearrange("p (c f) -> p c f", c=n),
                    in_=logits16[c : c + n].rearrange("c p f -> p c f"),
                )
                lg_dual[c] = t
                for k in range(n):
                    lg[c + k] = t[:, k * F : (k + 1) * F]

            # exp groups grow as the pipeline warms (amortizing ACT's 185ns
            # per-op cost), then shrink again near the tail so the PE matmuls
            # of the last groups don't bunch up and delay the stop matmuls
            GROUPS = [(0,), (1,), (2, 3), (4, 5), (6, 7, 8), (9, 10, 11),
                      (12, 13, 14, 15), (16, 17), (18, 19)]
            # enc rides late: the DVE gathers are tiny and have slack, while
            # ACT is arrival-paced at the head — the early slots go to lg
            for g in GROUPS[:7]:
                dma_lgn(g[0], len(g))
            nc.sync.dma_start(out=enc[:, :], in_=enc_in[:, :])
            for g in GROUPS[7:]:
                dma_lgn(g[0], len(g))
            # last channel in two half tiles so the tail chain is one half
            lg20 = [
                sp.tile([128, HB], dt.float16, name=f"lg20{h}", tag="lg20", bufs=2)
                for h in range(2)
            ]
            for h in range(2):
                nc.sync.dma_start(
                    out=lg20[h][:, :], in_=logits16[20][:, h * HB : (h + 1) * HB]
                )

            # ---- per-channel compute -------------------------------------
            wmax = max(e - s for rb in ranges for (s, e) in rb)

            def gather_dve(c, in1_tile, col0):
                # one pass per image half over class c's column range
                for b in range(2):
                    s, e = ranges[b][c]
                    tout = sp.tile(
                        [128, wmax], dt.float16, name="tout", tag="tout", bufs=2
                    )
                    nc.vector._custom_dve(
                        fused,
                        out=tout[:, : e - s],
                        in0=enc[:, s:e],
                        in1=in1_tile[:, s - col0 : e - col0],
                        s0=2.0 * c,
                        s1=0.5,
                        imm2=1.5,
                        accum_out=parts[:, 2 * c + b : 2 * c + b + 1],
                    )

            for gi, g in enumerate(GROUPS):
                n = len(g)
                exg = sp.tile(
                    [128, n * F], dt.bfloat16, name="ex", tag=f"ex{n}x", bufs=3
                )
                nc.scalar.activation(
                    out=exg[:, :], in_=lg_dual[g[0]][:, :], func=Act.Exp
                )
                for k, cc in enumerate(g):
                    exk = exg[:, k * F : (k + 1) * F]
                    for h in range(2):
                        nc.tensor.matmul(
                            dps[h][:, :],
                            ident[:, :],
                            exk[:, h * HB : (h + 1) * HB],
                            start=(cc == 0),
                            stop=False,
                        )
                    gather_dve(cc, lg[cc], 0)
            # c20: per-half exp -> stop matmul; image b's class-20 range sits
            # inside half-tile b (class 20 sorts just before the IGNORE block)
            assert ranges[0][20][1] <= HB and ranges[1][20][0] >= HB, ranges
            exh = []
            for h in range(2):
                e = sp.tile([128, HB], dt.bfloat16, name="exh", tag="exh", bufs=2)
                nc.scalar.activation(out=e[:, :], in_=lg20[h][:, :], func=Act.Exp)
                exh.append(e)
            for h in range(2):
                nc.tensor.matmul(
                    dps[h][:, :], ident[:, :], exh[h][:, :], start=False, stop=True
                )
            for b in range(2):
                s, e = ranges[b][20]
                tout = sp.tile(
                    [128, wmax], dt.float16, name="tout", tag="tout", bufs=2
                )
                nc.vector._custom_dve(
                    fused,
                    out=tout[:, : e - s],
                    in0=enc[:, s:e],
                    in1=lg20[b][:, s - b * HB : e - b * HB],
                    s0=2.0 * 20,
                    s1=0.5,
                    imm2=1.5,
                    accum_out=parts[:, 40 + b : 41 + b],
                )
            # partials out on SP before the den DMAs (it is ready earlier)
            nc.sync.dma_start(out=partials_out[:, :], in_=parts[:, :])
            # den copies run in parallel: bank 0 on ACT (right after its last
            # exp), bank 1 on the idle DVE; DMAs from the matching queues
            # (measured best of the engine/queue assignments)
            nc.scalar.activation(out=dsb[0][:, :], in_=dps[0][:, :], func=Act.Copy)
            nc.scalar.dma_start(out=den_out[:, 0:HB], in_=dsb[0][:, :])
            nc.vector.tensor_copy(out=dsb[1][:, :], in_=dps[1][:, :])
            nc.sync.dma_start(out=den_out[:, HB:F], in_=dsb[1][:, :])

    nc.compile()
    return nc


def _get_nc(ranges=None):
    if ranges is None:
        # introspection path (test harness): return the program built by the
        # latest kernel() call
        return next(reversed(_CACHE.values()))
    key = tuple(tuple(r) for r in (ranges[0] + ranges[1]))
    if key not in _CACHE:
        _CACHE[key] = _build_nc(ranges)
    return _CACHE[key]


def _gauss_1d():
    x = np.arange(2 * R + 1, dtype=np.float64) - R
    g = np.exp(-(x**2) / (2.0 * SIGMA**2))
    return (g / g.sum()).astype(np.float32)


def _host_gamma(bboxes):
    """Gamma weight maps [B,H,W] plus per-image Gamma sums; depends only on bboxes."""
    bb = bboxes.reshape(B * NB, 5).astype(np.int64)
    x0, y0, x1, y1, cls = bb[:, 0], bb[:, 1], bb[:, 2], bb[:, 3], bb[:, 4]
    valid = cls != -1
    ys = np.arange(H)
    xs = np.arange(W)
    row_m = (ys[None, :] >= y0[:, None]) & (ys[None, :] <= y1[:, None])  # [M,H]
    col_m = (xs[None, :] >= x0[:, None]) & (xs[None, :] <= x1[:, None])  # [M,W]
    in_r = (ys[None, :] > y0[:, None]) & (ys[None, :] < y1[:, None])
    in_c = (xs[None, :] > x0[:, None]) & (xs[None, :] < x1[:, None])

    nop = np.ones((B, H, W), dtype=np.float32)
    dis = np.zeros((B, H, W), dtype=np.float32)
    for m in range(B * NB):
        if not valid[m]:
            continue
        b = m // NB
        full = np.outer(row_m[m], col_m[m]).astype(np.float32)
        inner = np.outer(in_r[m], in_c[m]).astype(np.float32)
        nop[b] += full
        dis[b] += full * (1.0 - inner)

    g = _gauss_1d().astype(np.float64)
    # reflect-pad + separable 7x7 gaussian (matches conv with outer(g, g), 'VALID')
    disp = np.pad(dis, ((0, 0), (R, R), (0, 0)), mode="reflect").astype(np.float64)
    tmp = np.zeros((B, H, W), dtype=np.float64)
    for k in range(2 * R + 1):
        tmp += g[k] * disp[:, k : k + H, :]
    tmp = np.pad(tmp, ((0, 0), (0, 0), (R, R)), mode="reflect")
    blur = np.zeros((B, H, W), dtype=np.float64)
    for k in range(2 * R + 1):
        blur += g[k] * tmp[:, :, k : k + W]
    dis_b = blur.astype(np.float32) + 1.0

    nd = nop * dis_b
    ndmax = nd.max()
    sig = 1.0 / (1.0 + np.exp(-(nd / ndmax).astype(np.float64)))
    gam = ((sig - 0.5) * TAU + 1.0).astype(np.float32)
    s0 = gam.reshape(B, -1).astype(np.float64).sum(axis=1)  # per-image Gamma sums

    h = y1 - y0 + 1
    w = x1 - x0 + 1
    num_rc = 1e-5 + float(np.where(valid, h + w, 0).sum())
    return gam, s0, num_rc


def _host_box_terms(logits, bboxes, logden):
    """loss_rc from per-box window reductions on log-prob maps."""
    bb = bboxes.reshape(B * NB, 5).astype(np.int64)
    term = 0.0
    for m in range(B * NB):
        x0, y0, x1, y1, cls = bb[m]
        if cls == -1:
            continue
        b = m // NB
        lp = (
            logits[b, cls, y0 : y1 + 1, x0 : x1 + 1].astype(np.float64)
            - logden[b, y0 : y1 + 1, x0 : x1 + 1].astype(np.float64)
        )
        colmax = lp.max(axis=0)
        rowmax = lp.max(axis=1)
        colmin = lp.min(axis=0)
        rowmin = lp.min(axis=1)
        term += ALPHA * (colmax.sum() + rowmax.sum())
        term += (1.0 - ALPHA) * (
            np.log1p(-np.exp(colmin)).sum() + np.log1p(-np.exp(rowmin)).sum()
        )
    return -term


def _build_perms(labels):
    """Per-image label-sorted pixel permutations + per-(image-slot, class)
    absolute column ranges shared across cores. Within image b's column half,
    slot k (k = (col-512b)*128 + partition) holds pixel perm[k] of the
    image's flat [H*W] pixel index space."""
    perms = np.empty((B, IPIX), dtype=np.int64)
    Ks = np.empty((B, C + 2), dtype=np.int64)
    for i in range(B):
        lab = labels[i].reshape(-1)
        perms[i] = np.argsort(lab, kind="stable")
        cnt = np.bincount(np.minimum(lab, C), minlength=C + 1)
        Ks[i] = np.concatenate([[0], np.cumsum(cnt)])
    ranges = [[], []]
    for b in range(2):
        imgs = [i * IPC + b for i in range(N_CORES)]
        for c in range(C):
            s = min(int(Ks[i][c]) // 128 for i in imgs)
            e = max((int(Ks[i][c + 1]) - 1) // 128 + 1 for i in imgs)
            ranges[b].append((b * HB + s, b * HB + e))
    return perms, ranges


def _to_half(flat, perm):
    """flat [..., IPIX] pixel data -> [..., 128, HB] image-half layout."""
    a = np.take(flat, perm, axis=-1)
    a = a.reshape(*a.shape[:-1], HB, 128)
    return np.ascontiguousarray(np.swapaxes(a, -1, -2))


def _from_half(dev, perm):
    """[128, HB] image-half layout -> [IPIX] flat pixel data."""
    flat_k = np.ascontiguousarray(dev.T).reshape(-1)
    out = np.empty(IPIX, dtype=dev.dtype)
    out[perm] = flat_k
    return out


def kernel(logits, bboxes, labels):
    from concourse import bass_utils

    logits = np.asarray(logits, dtype=np.float32)
    bboxes = np.asarray(bboxes, dtype=np.int32)
    labels = np.asarray(labels, dtype=np.int32)

    gam, s0, num_rc = _host_gamma(bboxes)

    lab = labels.astype(np.float32)  # [B,H,W], IGNORE stays 255
    wmap = (labels != IGNORE).astype(np.float32) * gam
    enc = (2.0 * lab + wmap).astype(np.float16)  # [B,H,W]

    perms, ranges = _build_perms(labels)
    nc = _get_nc(ranges)

    lg16 = logits.astype(np.float16)  # [B,C,H,W]
    in_maps = []
    for i in range(N_CORES):
        img = [i * IPC, i * IPC + 1]
        # [C, 128, F]: image b's sorted pixels in columns [512b, 512b+512)
        lgi = np.concatenate(
            [
                _to_half(lg16[img[b]].reshape(C, IPIX), perms[img[b]])
                for b in range(2)
            ],
            axis=-1,
        )
        enci = np.concatenate(
            [_to_half(enc[img[b]].reshape(IPIX), perms[img[b]]) for b in range(2)],
            axis=-1,
        )
        in_maps.append({"logits16": lgi, "enc": enci})
    res = bass_utils.run_bass_kernel_spmd(nc, in_maps, core_ids=list(range(N_CORES)))

    logden = np.stack(
        [
            np.log(
                _from_half(
                    np.asarray(res.results[i // IPC]["den"])[
                        :, (i % IPC) * HB : (i % IPC + 1) * HB
                    ],
                    perms[i],
                ).astype(np.float32)
            ).reshape(H, W)
            for i in range(B)
        ],
        axis=0,
    )  # [B,H,W]
    loss_rc = _host_box_terms(logits, bboxes, logden)

    # weighted CE: sum w*logden (host, from exported map) - device gather sums
    wsum = (wmap.astype(np.float64) * logden.astype(np.float64)).reshape(B, -1).sum(axis=1)
    wce = 0.0
    for i in range(N_CORES):
        p = res.results[i]["partials"].astype(np.float64)
        for b in range(IPC):
            s1 = wsum[i * IPC + b] - p[:, b::2].sum()
            wce += s1 / s0[i * IPC + b]
    wce /= B

    out = LAMB * loss_rc / num_rc + wce
    return np.float32(out)


# revision 47
# speedup vs baseline: 1.0084x; 1.0042x over previous
"""Trainium2 Bass kernel for nn_Loss_PIP (PIP loss: box region terms + distance-map
weighted cross-entropy).

Strategy (data-parallel over batch across 8 NeuronCores, 2 images/core):
  - Pixel layout: the host deals each image's 65536 pixels into a
    [128 partitions x 512 column] half of the tile SORTED BY LABEL,
    round-robin across partitions (slot k -> partition k%128, column k//128;
    image b occupies columns [512b, 512b+512)). Every per-pixel computation
    (exp, denominator accumulation, den export, gather) is permutation-
    invariant, but the label-gather now only scans the narrow column range
    where class c of image b lives (~30 columns instead of 1024): the DVE
    gather drops from ~23.7us to ~4us and stops being the bottleneck. The
    class column ranges are computed from the actual labels at first call
    and compiled into the program (max across cores; stray neighbor-class
    pixels inside a range are killed by the enc gate).
  - Device (per core, SPMD): stream the 21 logit channel planes in fp16 (half
    the HBM bytes of f32); ACT computes exp (bf16, dual-channel ops to
    amortize fixed cost); PE accumulates the softmax denominator in PSUM via
    identity-matmul (identity built on-device on Pool); DVE runs one fused
    PIP_GATHER_DOT pass per (image, class) column range (driven by
    enc = 2*label + w) and the PSUM->SBUF den copies. The denominator is
    exported raw fp16; the host takes the log (no Ln => no activation-table
    switch). The last channel runs as two half planes to shorten the closing
    exp->matmul->copy->DMA chain; PSUM/staging tiles are split per bank to
    avoid false tile-granular WARs.
  - Host: Gamma weight-map pipeline (bbox-only), per-box window reductions on
    logden/logits, the w*logden reduction, permutation (un)packing, assembly.
"""

import sys

sys.path.insert(0, "/opt/trn_rl_repo")

import numpy as np

B, C, H, W = 16, 21, 256, 256
NB = 20
N_CORES = 8
IPC = B // N_CORES  # images per core
LAMB, ALPHA, TAU, R, SIGMA = 1.0, 0.5, 1.0, 3, 1.0
IGNORE = 255
F = 4 * W  # 1024 free elems per partition
HB = F // 2  # psum bank width in f32 = per-image column half
IPIX = H * W  # pixels per image = 128 * HB

_CACHE = {}


def _register_fused_op():
    """Register PIP_GATHER_DOT: out = m*(enc-s0)*in1, m = (enc-s0) in (s1, imm2);
    accum_out = sum(out). With enc = 2*label + w (w in {0} U (1,1.5)), s0=2c,
    s1=0.5, imm2=1.5 this computes w*(label==c)*logit in one DVE pass."""
    from concourse import dve_ops
    from concourse.dve_spec import C0, C1, C2, Spec, Src0, Src1, Zero, lower
    from concourse.dve_spec import _has_src1 as has_src1
    from concourse.dve_uop import DveOpSpec
    from operator import add as op_add
    import numpy as np_

    name = "PIP_GATHER_DOT"
    if name in dve_ops._SUB_OPCODE_FOR_NAME:
        return next(o for o in dve_ops.OPS if o.name == name)

    _t = Src0 - C0

    def _ref(in0, in1, s0, s1, imm2):
        t = in0.astype(np_.float32) - s0
        m = ((t > s1) & (t < imm2)).astype(np_.float32)
        b = (m * t * in1).astype(np_.float32)
        return b, b.reshape(b.shape[0], -1).sum(axis=-1, keepdims=True)

    spec = Spec(
        body=((_t > C1) & (_t < C2)) * _t * Src1,
        accum=op_add,
        accum_init=Zero,
        reference=_ref,
    )
    row = dve_ops._CUSTOM_DVE_ROW_BASE + len(dve_ops.OPS)
    assert row < 0x20
    shas = {}
    for ver in ("v3", "v4"):
        try:
            uops = lower(spec, ver=ver)
        except Exception:
            continue
        shas[ver] = DveOpSpec(
            name=name, opcode=row, uops=uops, rd1_en=has_src1(spec)
        ).sha(ver)
    op = dve_ops.DveOp(name, spec, subdim=False, uops_sha=shas)
    dve_ops.OPS.append(op)
    dve_ops.CUSTOM_DVE_SPECS[name] = spec
    dve_ops._SUB_OPCODE_FOR_NAME[name] = row
    return op


def _build_nc(ranges):
    """ranges[b][c] = (start, end) absolute column range (image b's half)
    covering every partition's class-c pixels across all cores."""
    import concourse.bacc as bacc
    import concourse.mybir as mybir
    from concourse import tile

    dt = mybir.dt
    Alu = mybir.AluOpType
    Act = mybir.ActivationFunctionType

    nc = bacc.Bacc(
        "TRN2",
        target_bir_lowering=False,
        debug=False,
        enable_asserts=False,
        num_devices=N_CORES,
    )

    # host supplies logits label-sorted + fp16: [c, p, f] = logits[pix(p,f)]
    logits16 = nc.dram_tensor("logits16", [C, 128, F], dt.float16, kind="ExternalInput")
    enc_in = nc.dram_tensor("enc", [128, F], dt.float16, kind="ExternalInput")
    den_out = nc.dram_tensor("den", [128, F], dt.float16, kind="ExternalOutput")
    # parts col 2c+b = sum_p w*(label==c)*logit_c over image b
    partials_out = nc.dram_tensor(
        "partials", [128, 2 * C], dt.float32, kind="ExternalOutput"
    )

    fused = _register_fused_op()

    with tile.TileContext(nc) as tc:
        with (
            tc.tile_pool(name="persist", bufs=1) as pp,
            tc.tile_pool(name="stream", bufs=4) as sp,
            tc.tile_pool(name="psum", bufs=1, space="PSUM") as psp,
        ):
            enc = pp.tile([128, F], dt.float16, name="enc")
            ident = pp.tile([128, 128], dt.bfloat16, name="ident")
            ones = pp.tile([128, 128], dt.bfloat16, name="ones")
            parts = pp.tile([128, 2 * C], dt.float32, name="parts")
            # separate PSUM/SBUF tiles per bank half: no false WAR between the
            # bank-0 epilogue and bank-1 accumulation
            dps = [psp.tile([128, HB], dt.float32, name=f"dps{h}") for h in range(2)]
            dsb = [pp.tile([128, HB], dt.float16, name=f"dsb{h}") for h in range(2)]

            # identity for the PE accumulate, generated on the Pool engine
            nc.gpsimd.memset(ones[:, :], 1.0)
            nc.gpsimd.affine_select(
                out=ident[:, :],
                in_=ones[:, :],
                pattern=[[1, 128]],
                compare_op=Alu.is_equal,
                fill=0.0,
                base=0,
                channel_multiplier=-1,
            )

            # ---- input stream on the SP queue ----------------------------
            lg = {}
            lg_dual = {}

            def dma_lg(c):
                t = sp.tile([128, F], dt.float16, name=f"lg{c}", tag="lg", bufs=C)
                nc.sync.dma_start(out=t[:, :], in_=logits16[c])
                lg[c] = t

            def dma_lgn(c, n):  # channels c..c+n-1 in one DMA / one tile
                t = sp.tile(
                    [128, n * F], dt.float16, name=f"lg{c}", tag=f"lg{n}x", bufs=4
                )
                nc.sync.dma_start(
                    out=t[:, :].rearrange("p (c f) -> p c f", c=n),
                    in_=logits16[c : c + n].rearrange("c p f -> p c f"),
                )
                lg_dual[c] = t
                for k in range(n):
                    lg[c + k] = t[:, k * F : (k + 1) * F]

            # exp groups grow as the pipeline warms (amortizing ACT's 185ns
            # per-op cost), then shrink again near the tail so the PE matmuls
            # of the last groups don't bunch up and delay the stop matmuls
            GROUPS = [(0,), (1,), (2, 3), (4, 5), (6, 7, 8), (9, 10, 11),
                      (12, 13, 14, 15), (16, 17), (18, 19)]
            # enc rides late: the DVE gathers are tiny and have slack, while
            # ACT is arrival-paced at the head — the early slots go to lg
            for g in GROUPS[:7]:
                dma_lgn(g[0], len(g))
            nc.sync.dma_start(out=enc[:, :], in_=enc_in[:, :])
            for g in GROUPS[7:]:
                dma_lgn(g[0], len(g))
            # last channel in two half tiles so the tail chain is one half
            lg20 = [
                sp.tile([128, HB], dt.float16, name=f"lg20{h}", tag="lg20", bufs=2)
                for h in range(2)
            ]
            for h in range(2):
                nc.sync.dma_start(
                    out=lg20[h][:, :], in_=logits16[20][:, h * HB : (h + 1) * HB]
                )

            # ---- per-channel compute -------------------------------------
            wmax = max(e - s for rb in ranges for (s, e) in rb)

            def gather_dve(c, in1_tile, col0):
                # one pass per image half over class c's column range
                for b in range(2):
                    s, e = ranges[b][c]
                    tout = sp.tile(
                        [128, wmax], dt.float16, name="tout", tag="tout", bufs=2
                    )
                    nc.vector._custom_dve(
                        fused,
                        out=tout[:, : e - s],
                        in0=enc[:, s:e],
                        in1=in1_tile[:, s - col0 : e - col0],
                        s0=2.0 * c,
                        s1=0.5,
                        imm2=1.5,
                        accum_out=parts[:, 2 * c + b : 2 * c + b + 1],
                    )

            last_bank1 = []  # (ex-slice) of the final group, deferred

            for gi, g in enumerate(GROUPS):
                n = len(g)
                last = gi == len(GROUPS) - 1
                exg = sp.tile(
                    [128, n * F], dt.bfloat16, name="ex", tag=f"ex{n}x", bufs=3
                )
                nc.scalar.activation(
                    out=exg[:, :], in_=lg_dual[g[0]][:, :], func=Act.Exp
                )
                for k, cc in enumerate(g):
                    exk = exg[:, k * F : (k + 1) * F]
                    for h in range(2):
                        if last and h == 1:
                            # defer the final group's bank1 matmuls past the
                            # bank0 stop so mm_a (and the den_a path) isn't
                            # stuck behind this group's matmul bunch
                            last_bank1.append(exk)
                            continue
                        nc.tensor.matmul(
                            dps[h][:, :],
                            ident[:, :],
                            exk[:, h * HB : (h + 1) * HB],
                            start=(cc == 0),
                            stop=False,
                        )
                    gather_dve(cc, lg[cc], 0)
            # c20: per-half exp -> stop matmul; image b's class-20 range sits
            # inside half-tile b (class 20 sorts just before the IGNORE block)
            assert ranges[0][20][1] <= HB and ranges[1][20][0] >= HB, ranges
            exh = []
            for h in range(2):
                e = sp.tile([128, HB], dt.bfloat16, name="exh", tag="exh", bufs=2)
                nc.scalar.activation(out=e[:, :], in_=lg20[h][:, :], func=Act.Exp)
                exh.append(e)
            nc.tensor.matmul(
                dps[0][:, :], ident[:, :], exh[0][:, :], start=False, stop=True
            )
            for exk in last_bank1:
                nc.tensor.matmul(
                    dps[1][:, :], ident[:, :], exk[:, HB:F], start=False, stop=False
                )
            nc.tensor.matmul(
                dps[1][:, :], ident[:, :], exh[1][:, :], start=False, stop=True
            )
            for b in range(2):
                s, e = ranges[b][20]
                tout = sp.tile(
                    [128, wmax], dt.float16, name="tout", tag="tout", bufs=2
                )
                nc.vector._custom_dve(
                    fused,
                    out=tout[:, : e - s],
                    in0=enc[:, s:e],
                    in1=lg20[b][:, s - b * HB : e - b * HB],
                    s0=2.0 * 20,
                    s1=0.5,
                    imm2=1.5,
                    accum_out=parts[:, 40 + b : 41 + b],
                )
            # partials out on SP before the den DMAs (it is ready earlier)
            nc.sync.dma_start(out=partials_out[:, :], in_=parts[:, :])
            # den copies run in parallel: bank 0 on ACT (right after its last
            # exp), bank 1 on the idle DVE; DMAs from the matching queues
            # (measured best of the engine/queue assignments)
            nc.scalar.activation(out=dsb[0][:, :], in_=dps[0][:, :], func=Act.Copy)
            nc.scalar.dma_start(out=den_out[:, 0:HB], in_=dsb[0][:, :])
            nc.vector.tensor_copy(out=dsb[1][:, :], in_=dps[1][:, :])
            nc.sync.dma_start(out=den_out[:, HB:F], in_=dsb[1][:, :])

    nc.compile()
    return nc


def _get_nc(ranges=None):
    if ranges is None:
        # introspection path (test harness): return the program built by the
        # latest kernel() call
        return next(reversed(_CACHE.values()))
    key = tuple(tuple(r) for r in (ranges[0] + ranges[1]))
    if key not in _CACHE:
        _CACHE[key] = _build_nc(ranges)
    return _CACHE[key]


def _gauss_1d():
    x = np.arange(2 * R + 1, dtype=np.float64) - R
    g = np.exp(-(x**2) / (2.0 * SIGMA**2))
    return (g / g.sum()).astype(np.float32)


def _host_gamma(bboxes):
    """Gamma weight maps [B,H,W] plus per-image Gamma sums; depends only on bboxes."""
    bb = bboxes.reshape(B * NB, 5).astype(np.int64)
    x0, y0, x1, y1, cls = bb[:, 0], bb[:, 1], bb[:, 2], bb[:, 3], bb[:, 4]
    valid = cls != -1
    ys = np.arange(H)
    xs = np.arange(W)
    row_m = (ys[None, :] >= y0[:, None]) & (ys[None, :] <= y1[:, None])  # [M,H]
    col_m = (xs[None, :] >= x0[:, None]) & (xs[None, :] <= x1[:, None])  # [M,W]
    in_r = (ys[None, :] > y0[:, None]) & (ys[None, :] < y1[:, None])
    in_c = (xs[None, :] > x0[:, None]) & (xs[None, :] < x1[:, None])

    nop = np.ones((B, H, W), dtype=np.float32)
    dis = np.zeros((B, H, W), dtype=np.float32)
    for m in range(B * NB):
        if not valid[m]:
            continue
        b = m // NB
        full = np.outer(row_m[m], col_m[m]).astype(np.float32)
        inner = np.outer(in_r[m], in_c[m]).astype(np.float32)
        nop[b] += full
        dis[b] += full * (1.0 - inner)

    g = _gauss_1d().astype(np.float64)
    # reflect-pad + separable 7x7 gaussian (matches conv with outer(g, g), 'VALID')
    disp = np.pad(dis, ((0, 0), (R, R), (0, 0)), mode="reflect").astype(np.float64)
    tmp = np.zeros((B, H, W), dtype=np.float64)
    for k in range(2 * R + 1):
        tmp += g[k] * disp[:, k : k + H, :]
    tmp = np.pad(tmp, ((0, 0), (0, 0), (R, R)), mode="reflect")
    blur = np.zeros((B, H, W), dtype=np.float64)
    for k in range(2 * R + 1):
        blur += g[k] * tmp[:, :, k : k + W]
    dis_b = blur.astype(np.float32) + 1.0

    nd = nop * dis_b
    ndmax = nd.max()
    sig = 1.0 / (1.0 + np.exp(-(nd / ndmax).astype(np.float64)))
    gam = ((sig - 0.5) * TAU + 1.0).astype(np.float32)
    s0 = gam.reshape(B, -1).astype(np.float64).sum(axis=1)  # per-image Gamma sums

    h = y1 - y0 + 1
    w = x1 - x0 + 1
    num_rc = 1e-5 + float(np.where(valid, h + w, 0).sum())
    return gam, s0, num_rc


def _host_box_terms(logits, bboxes, logden):
    """loss_rc from per-box window reductions on log-prob maps."""
    bb = bboxes.reshape(B * NB, 5).astype(np.int64)
    term = 0.0
    for m in range(B * NB):
        x0, y0, x1, y1, cls = bb[m]
        if cls == -1:
            continue
        b = m // NB
        lp = (
            logits[b, cls, y0 : y1 + 1, x0 : x1 + 1].astype(np.float64)
            - logden[b, y0 : y1 + 1, x0 : x1 + 1].astype(np.float64)
        )
        colmax = lp.max(axis=0)
        rowmax = lp.max(axis=1)
        colmin = lp.min(axis=0)
        rowmin = lp.min(axis=1)
        term += ALPHA * (colmax.sum() + rowmax.sum())
        term += (1.0 - ALPHA) * (
            np.log1p(-np.exp(colmin)).sum() + np.log1p(-np.exp(rowmin)).sum()
        )
    return -term


def _build_perms(labels):
    """Per-image label-sorted pixel permutations + per-(image-slot, class)
    absolute column ranges shared across cores. Within image b's column half,
    slot k (k = (col-512b)*128 + partition) holds pixel perm[k] of the
    image's flat [H*W] pixel index space."""
    perms = np.empty((B, IPIX), dtype=np.int64)
    Ks = np.empty((B, C + 2), dtype=np.int64)
    for i in range(B):
        lab = labels[i].reshape(-1)
        perms[i] = np.argsort(lab, kind="stable")
        cnt = np.bincount(np.minimum(lab, C), minlength=C + 1)
        Ks[i] = np.concatenate([[0], np.cumsum(cnt)])
    ranges = [[], []]
    for b in range(2):
        imgs = [i * IPC + b for i in range(N_CORES)]
        for c in range(C):
            s = min(int(Ks[i][c]) // 128 for i in imgs)
            e = max((int(Ks[i][c + 1]) - 1) // 128 + 1 for i in imgs)
            ranges[b].append((b * HB + s, b * HB + e))
    return perms, ranges


def _to_half(flat, perm):
    """flat [..., IPIX] pixel data -> [..., 128, HB] image-half layout."""
    a = np.take(flat, perm, axis=-1)
    a = a.reshape(*a.shape[:-1], HB, 128)
    return np.ascontiguousarray(np.swapaxes(a, -1, -2))


def _from_half(dev, perm):
    """[128, HB] image-half layout -> [IPIX] flat pixel data."""
    flat_k = np.ascontiguousarray(dev.T).reshape(-1)
    out = np.empty(IPIX, dtype=dev.dtype)
    out[perm] = flat_k
    return out


def kernel(logits, bboxes, labels):
    from concourse import bass_utils

    logits = np.asarray(logits, dtype=np.float32)
    bboxes = np.asarray(bboxes, dtype=np.int32)
    labels = np.asarray(labels, dtype=np.int32)

    gam, s0, num_rc = _host_gamma(bboxes)

    lab = labels.astype(np.float32)  # [B,H,W], IGNORE stays 255
    wmap = (labels != IGNORE).astype(np.float32) * gam
    enc = (2.0 * lab + wmap).astype(np.float16)  # [B,H,W]

    perms, ranges = _build_perms(labels)
    nc = _get_nc(ranges)

    lg16 = logits.astype(np.float16)  # [B,C,H,W]
    in_maps = []
    for i in range(N_CORES):
        img = [i * IPC, i * IPC + 1]
        # [C, 128, F]: image b's sorted pixels in columns [512b, 512b+512)
        lgi = np.concatenate(
            [
                _to_half(lg16[img[b]].reshape(C, IPIX), perms[img[b]])
                for b in range(2)
            ],
            axis=-1,
        )
        enci = np.concatenate(
            [_to_half(enc[img[b]].reshape(IPIX), perms[img[b]]) for b in range(2)],
            axis=-1,
        )
        in_maps.append({"logits16": lgi, "enc": enci})
    res = bass_utils.run_bass_kernel_spmd(nc, in_maps, core_ids=list(range(N_CORES)))

    logden = np.stack(
        [
            np.log(
                _from_half(
                    np.asarray(res.results[i // IPC]["den"])[
                        :, (i % IPC) * HB : (i % IPC + 1) * HB
                    ],
                    perms[i],
                ).astype(np.float32)
            ).reshape(H, W)
            for i in range(B)
        ],
        axis=0,
    )  # [B,H,W]
    loss_rc = _host_box_terms(logits, bboxes, logden)

    # weighted CE: sum w*logden (host, from exported map) - device gather sums
    wsum = (wmap.astype(np.float64) * logden.astype(np.float64)).reshape(B, -1).sum(axis=1)
    wce = 0.0
    for i in range(N_CORES):
        p = res.results[i]["partials"].astype(np.float64)
        for b in range(IPC):
            s1 = wsum[i * IPC + b] - p[:, b::2].sum()
            wce += s1 / s0[i * IPC + b]
    wce /= B

    out = LAMB * loss_rc / num_rc + wce
    return np.float32(out)


# revision 48
# speedup vs baseline: 1.0114x; 1.0030x over previous
"""Trainium2 Bass kernel for nn_Loss_PIP (PIP loss: box region terms + distance-map
weighted cross-entropy).

Strategy (data-parallel over batch across 8 NeuronCores, 2 images/core):
  - Pixel layout: the host deals each image's 65536 pixels into a
    [128 partitions x 512 column] half of the tile SORTED BY LABEL,
    round-robin across partitions (slot k -> partition k%128, column k//128;
    image b occupies columns [512b, 512b+512)). Every per-pixel computation
    (exp, denominator accumulation, den export, gather) is permutation-
    invariant, but the label-gather now only scans the narrow column range
    where class c of image b lives (~30 columns instead of 1024): the DVE
    gather drops from ~23.7us to ~4us and stops being the bottleneck. The
    class column ranges are computed from the actual labels at first call
    and compiled into the program (max across cores; stray neighbor-class
    pixels inside a range are killed by the enc gate).
  - Device (per core, SPMD): stream the 21 logit channel planes in fp16 (half
    the HBM bytes of f32); ACT computes exp (bf16, dual-channel ops to
    amortize fixed cost); PE accumulates the softmax denominator in PSUM via
    identity-matmul (identity built on-device on Pool); DVE runs one fused
    PIP_GATHER_DOT pass per (image, class) column range (driven by
    enc = 2*label + w) and the PSUM->SBUF den copies. The denominator is
    exported raw fp16; the host takes the log (no Ln => no activation-table
    switch). The last channel runs as two half planes to shorten the closing
    exp->matmul->copy->DMA chain; PSUM/staging tiles are split per bank to
    avoid false tile-granular WARs.
  - Host: Gamma weight-map pipeline (bbox-only), per-box window reductions on
    logden/logits, the w*logden reduction, permutation (un)packing, assembly.
"""

import sys

sys.path.insert(0, "/opt/trn_rl_repo")

import numpy as np

B, C, H, W = 16, 21, 256, 256
NB = 20
N_CORES = 8
IPC = B // N_CORES  # images per core
LAMB, ALPHA, TAU, R, SIGMA = 1.0, 0.5, 1.0, 3, 1.0
IGNORE = 255
F = 4 * W  # 1024 free elems per partition
HB = F // 2  # psum bank width in f32 = per-image column half
IPIX = H * W  # pixels per image = 128 * HB

_CACHE = {}


def _register_fused_op():
    """Register PIP_GATHER_DOT: out = m*(enc-s0)*in1, m = (enc-s0) in (s1, imm2);
    accum_out = sum(out). With enc = 2*label + w (w in {0} U (1,1.5)), s0=2c,
    s1=0.5, imm2=1.5 this computes w*(label==c)*logit in one DVE pass."""
    from concourse import dve_ops
    from concourse.dve_spec import C0, C1, C2, Spec, Src0, Src1, Zero, lower
    from concourse.dve_spec import _has_src1 as has_src1
    from concourse.dve_uop import DveOpSpec
    from operator import add as op_add
    import numpy as np_

    name = "PIP_GATHER_DOT"
    if name in dve_ops._SUB_OPCODE_FOR_NAME:
        return next(o for o in dve_ops.OPS if o.name == name)

    _t = Src0 - C0

    def _ref(in0, in1, s0, s1, imm2):
        t = in0.astype(np_.float32) - s0
        m = ((t > s1) & (t < imm2)).astype(np_.float32)
        b = (m * t * in1).astype(np_.float32)
        return b, b.reshape(b.shape[0], -1).sum(axis=-1, keepdims=True)

    spec = Spec(
        body=((_t > C1) & (_t < C2)) * _t * Src1,
        accum=op_add,
        accum_init=Zero,
        reference=_ref,
    )
    row = dve_ops._CUSTOM_DVE_ROW_BASE + len(dve_ops.OPS)
    assert row < 0x20
    shas = {}
    for ver in ("v3", "v4"):
        try:
            uops = lower(spec, ver=ver)
        except Exception:
            continue
        shas[ver] = DveOpSpec(
            name=name, opcode=row, uops=uops, rd1_en=has_src1(spec)
        ).sha(ver)
    op = dve_ops.DveOp(name, spec, subdim=False, uops_sha=shas)
    dve_ops.OPS.append(op)
    dve_ops.CUSTOM_DVE_SPECS[name] = spec
    dve_ops._SUB_OPCODE_FOR_NAME[name] = row
    return op


def _build_nc(ranges):
    """ranges[b][c] = (start, end) absolute column range (image b's half)
    covering every partition's class-c pixels across all cores."""
    import concourse.bacc as bacc
    import concourse.mybir as mybir
    from concourse import tile

    dt = mybir.dt
    Alu = mybir.AluOpType
    Act = mybir.ActivationFunctionType

    nc = bacc.Bacc(
        "TRN2",
        target_bir_lowering=False,
        debug=False,
        enable_asserts=False,
        num_devices=N_CORES,
    )

    # host supplies logits label-sorted + fp16: [c, p, f] = logits[pix(p,f)]
    logits16 = nc.dram_tensor("logits16", [C, 128, F], dt.float16, kind="ExternalInput")
    enc_in = nc.dram_tensor("enc", [128, F], dt.float16, kind="ExternalInput")
    den_out = nc.dram_tensor("den", [128, F], dt.float16, kind="ExternalOutput")
    # parts col 2c+b = sum_p w*(label==c)*logit_c over image b
    partials_out = nc.dram_tensor(
        "partials", [128, 2 * C], dt.float32, kind="ExternalOutput"
    )

    fused = _register_fused_op()

    with tile.TileContext(nc) as tc:
        with (
            tc.tile_pool(name="persist", bufs=1) as pp,
            tc.tile_pool(name="stream", bufs=4) as sp,
            tc.tile_pool(name="psum", bufs=1, space="PSUM") as psp,
        ):
            enc = pp.tile([128, F], dt.float16, name="enc")
            ident = pp.tile([128, 128], dt.bfloat16, name="ident")
            ones = pp.tile([128, 128], dt.bfloat16, name="ones")
            parts = pp.tile([128, 2 * C], dt.float32, name="parts")
            # separate PSUM/SBUF tiles per bank half: no false WAR between the
            # bank-0 epilogue and bank-1 accumulation
            dps = [psp.tile([128, HB], dt.float32, name=f"dps{h}") for h in range(2)]
            dsb = [pp.tile([128, HB], dt.float16, name=f"dsb{h}") for h in range(2)]

            # identity for the PE accumulate, generated on the Pool engine
            nc.gpsimd.memset(ones[:, :], 1.0)
            nc.gpsimd.affine_select(
                out=ident[:, :],
                in_=ones[:, :],
                pattern=[[1, 128]],
                compare_op=Alu.is_equal,
                fill=0.0,
                base=0,
                channel_multiplier=-1,
            )

            # ---- input stream on the SP queue ----------------------------
            lg = {}
            lg_dual = {}

            def dma_lg(c):
                t = sp.tile([128, F], dt.float16, name=f"lg{c}", tag="lg", bufs=C)
                nc.sync.dma_start(out=t[:, :], in_=logits16[c])
                lg[c] = t

            def dma_lgn(c, n):  # channels c..c+n-1 in one DMA / one tile
                t = sp.tile(
                    [128, n * F], dt.float16, name=f"lg{c}", tag=f"lg{n}x", bufs=4
                )
                nc.sync.dma_start(
                    out=t[:, :].rearrange("p (c f) -> p c f", c=n),
                    in_=logits16[c : c + n].rearrange("c p f -> p c f"),
                )
                lg_dual[c] = t
                for k in range(n):
                    lg[c + k] = t[:, k * F : (k + 1) * F]

            # exp groups grow as the pipeline warms (amortizing ACT's 185ns
            # per-op cost), then shrink again near the tail so the PE matmuls
            # of the last groups don't bunch up and delay the stop matmuls
            GROUPS = [(0,), (1,), (2, 3), (4, 5), (6, 7, 8), (9, 10, 11),
                      (12, 13, 14, 15), (16, 17), (18, 19)]
            # enc rides late: the DVE gathers are tiny and have slack, while
            # ACT is arrival-paced at the head — the early slots go to lg
            for g in GROUPS[:7]:
                dma_lgn(g[0], len(g))
            nc.sync.dma_start(out=enc[:, :], in_=enc_in[:, :])
            for g in GROUPS[7:]:
                dma_lgn(g[0], len(g))
            # last channel in two half tiles so the tail chain is one half
            lg20 = [
                sp.tile([128, HB], dt.float16, name=f"lg20{h}", tag="lg20", bufs=2)
                for h in range(2)
            ]
            for h in range(2):
                nc.sync.dma_start(
                    out=lg20[h][:, :], in_=logits16[20][:, h * HB : (h + 1) * HB]
                )

            # ---- per-channel compute -------------------------------------
            wmax = max(e - s for rb in ranges for (s, e) in rb)

            def gather_dve(c, in1_tile, col0):
                # one pass per image half over class c's column range
                for b in range(2):
                    s, e = ranges[b][c]
                    tout = sp.tile(
                        [128, wmax], dt.float16, name="tout", tag="tout", bufs=2
                    )
                    nc.vector._custom_dve(
                        fused,
                        out=tout[:, : e - s],
                        in0=enc[:, s:e],
                        in1=in1_tile[:, s - col0 : e - col0],
                        s0=2.0 * c,
                        s1=0.5,
                        imm2=1.5,
                        accum_out=parts[:, 2 * c + b : 2 * c + b + 1],
                    )

            last_bank1 = []  # (ex-slice) of the final group, deferred

            for gi, g in enumerate(GROUPS):
                n = len(g)
                last = gi == len(GROUPS) - 1
                exg = sp.tile(
                    [128, n * F], dt.bfloat16, name="ex", tag=f"ex{n}x", bufs=3
                )
                nc.scalar.activation(
                    out=exg[:, :], in_=lg_dual[g[0]][:, :], func=Act.Exp
                )
                for k, cc in enumerate(g):
                    exk = exg[:, k * F : (k + 1) * F]
                    for h in range(2):
                        if last and h == 1:
                            # defer the final group's bank1 matmuls past the
                            # bank0 stop so mm_a (and the den_a path) isn't
                            # stuck behind this group's matmul bunch
                            last_bank1.append(exk)
                            continue
                        nc.tensor.matmul(
                            dps[h][:, :],
                            ident[:, :],
                            exk[:, h * HB : (h + 1) * HB],
                            start=(cc == 0),
                            stop=False,
                        )
                    gather_dve(cc, lg[cc], 0)
            # c20: per-half exp -> stop matmul; image b's class-20 range sits
            # inside half-tile b (class 20 sorts just before the IGNORE block)
            assert ranges[0][20][1] <= HB and ranges[1][20][0] >= HB, ranges
            exh = []
            for h in range(2):
                e = sp.tile([128, HB], dt.bfloat16, name="exh", tag="exh", bufs=2)
                nc.scalar.activation(out=e[:, :], in_=lg20[h][:, :], func=Act.Exp)
                exh.append(e)
            nc.tensor.matmul(
                dps[0][:, :], ident[:, :], exh[0][:, :], start=False, stop=True
            )
            for exk in last_bank1:
                nc.tensor.matmul(
                    dps[1][:, :], ident[:, :], exk[:, HB:F], start=False, stop=False
                )
            nc.tensor.matmul(
                dps[1][:, :], ident[:, :], exh[1][:, :], start=False, stop=True
            )
            for b in range(2):
                s, e = ranges[b][20]
                tout = sp.tile(
                    [128, wmax], dt.float16, name="tout", tag="tout", bufs=2
                )
                nc.vector._custom_dve(
                    fused,
                    out=tout[:, : e - s],
                    in0=enc[:, s:e],
                    in1=lg20[b][:, s - b * HB : e - b * HB],
                    s0=2.0 * 20,
                    s1=0.5,
                    imm2=1.5,
                    accum_out=parts[:, 40 + b : 41 + b],
                )
            # partials out on SP before the den DMAs (it is ready earlier)
            nc.sync.dma_start(out=partials_out[:, :], in_=parts[:, :])
            # both den copies on the idle DVE: copy_a right after mm_a (not
            # stuck behind ACT's remaining exps), so den_a's HWDGE descriptor
            # gen clears the shared HWDGE stage before den_b's gen needs it;
            # copy_b still starts at mm_b+sem either way. den_a DMA issues
            # from the (exp-free) ACT queue, den_b from SP.
            nc.vector.tensor_copy(out=dsb[0][:, :], in_=dps[0][:, :])
            nc.scalar.dma_start(out=den_out[:, 0:HB], in_=dsb[0][:, :])
            nc.vector.tensor_copy(out=dsb[1][:, :], in_=dps[1][:, :])
            nc.sync.dma_start(out=den_out[:, HB:F], in_=dsb[1][:, :])

    nc.compile()
    return nc


def _get_nc(ranges=None):
    if ranges is None:
        # introspection path (test harness): return the program built by the
        # latest kernel() call
        return next(reversed(_CACHE.values()))
    key = tuple(tuple(r) for r in (ranges[0] + ranges[1]))
    if key not in _CACHE:
        _CACHE[key] = _build_nc(ranges)
    return _CACHE[key]


def _gauss_1d():
    x = np.arange(2 * R + 1, dtype=np.float64) - R
    g = np.exp(-(x**2) / (2.0 * SIGMA**2))
    return (g / g.sum()).astype(np.float32)


def _host_gamma(bboxes):
    """Gamma weight maps [B,H,W] plus per-image Gamma sums; depends only on bboxes."""
    bb = bboxes.reshape(B * NB, 5).astype(np.int64)
    x0, y0, x1, y1, cls = bb[:, 0], bb[:, 1], bb[:, 2], bb[:, 3], bb[:, 4]
    valid = cls != -1
    ys = np.arange(H)
    xs = np.arange(W)
    row_m = (ys[None, :] >= y0[:, None]) & (ys[None, :] <= y1[:, None])  # [M,H]
    col_m = (xs[None, :] >= x0[:, None]) & (xs[None, :] <= x1[:, None])  # [M,W]
    in_r = (ys[None, :] > y0[:, None]) & (ys[None, :] < y1[:, None])
    in_c = (xs[None, :] > x0[:, None]) & (xs[None, :] < x1[:, None])

    nop = np.ones((B, H, W), dtype=np.float32)
    dis = np.zeros((B, H, W), dtype=np.float32)
    for m in range(B * NB):
        if not valid[m]:
            continue
        b = m // NB
        full = np.outer(row_m[m], col_m[m]).astype(np.float32)
        inner = np.outer(in_r[m], in_c[m]).astype(np.float32)
        nop[b] += full
        dis[b] += full * (1.0 - inner)

    g = _gauss_1d().astype(np.float64)
    # reflect-pad + separable 7x7 gaussian (matches conv with outer(g, g), 'VALID')
    disp = np.pad(dis, ((0, 0), (R, R), (0, 0)), mode="reflect").astype(np.float64)
    tmp = np.zeros((B, H, W), dtype=np.float64)
    for k in range(2 * R + 1):
        tmp += g[k] * disp[:, k : k + H, :]
    tmp = np.pad(tmp, ((0, 0), (0, 0), (R, R)), mode="reflect")
    blur = np.zeros((B, H, W), dtype=np.float64)
    for k in range(2 * R + 1):
        blur += g[k] * tmp[:, :, k : k + W]
    dis_b = blur.astype(np.float32) + 1.0

    nd = nop * dis_b
    ndmax = nd.max()
    sig = 1.0 / (1.0 + np.exp(-(nd / ndmax).astype(np.float64)))
    gam = ((sig - 0.5) * TAU + 1.0).astype(np.float32)
    s0 = gam.reshape(B, -1).astype(np.float64).sum(axis=1)  # per-image Gamma sums

    h = y1 - y0 + 1
    w = x1 - x0 + 1
    num_rc = 1e-5 + float(np.where(valid, h + w, 0).sum())
    return gam, s0, num_rc


def _host_box_terms(logits, bboxes, logden):
    """loss_rc from per-box window reductions on log-prob maps."""
    bb = bboxes.reshape(B * NB, 5).astype(np.int64)
    term = 0.0
    for m in range(B * NB):
        x0, y0, x1, y1, cls = bb[m]
        if cls == -1:
            continue
        b = m // NB
        lp = (
            logits[b, cls, y0 : y1 + 1, x0 : x1 + 1].astype(np.float64)
            - logden[b, y0 : y1 + 1, x0 : x1 + 1].astype(np.float64)
        )
        colmax = lp.max(axis=0)
        rowmax = lp.max(axis=1)
        colmin = lp.min(axis=0)
        rowmin = lp.min(axis=1)
        term += ALPHA * (colmax.sum() + rowmax.sum())
        term += (1.0 - ALPHA) * (
            np.log1p(-np.exp(colmin)).sum() + np.log1p(-np.exp(rowmin)).sum()
        )
    return -term


def _build_perms(labels):
    """Per-image label-sorted pixel permutations + per-(image-slot, class)
    absolute column ranges shared across cores. Within image b's column half,
    slot k (k = (col-512b)*128 + partition) holds pixel perm[k] of the
    image's flat [H*W] pixel index space."""
    perms = np.empty((B, IPIX), dtype=np.int64)
    Ks = np.empty((B, C + 2), dtype=np.int64)
    for i in range(B):
        lab = labels[i].reshape(-1)
        perms[i] = np.argsort(lab, kind="stable")
        cnt = np.bincount(np.minimum(lab, C), minlength=C + 1)
        Ks[i] = np.concatenate([[0], np.cumsum(cnt)])
    ranges = [[], []]
    for b in range(2):
        imgs = [i * IPC + b for i in range(N_CORES)]
        for c in range(C):
            s = min(int(Ks[i][c]) // 128 for i in imgs)
            e = max((int(Ks[i][c + 1]) - 1) // 128 + 1 for i in imgs)
            ranges[b].append((b * HB + s, b * HB + e))
    return perms, ranges


def _to_half(flat, perm):
    """flat [..., IPIX] pixel data -> [..., 128, HB] image-half layout."""
    a = np.take(flat, perm, axis=-1)
    a = a.reshape(*a.shape[:-1], HB, 128)
    return np.ascontiguousarray(np.swapaxes(a, -1, -2))


def _from_half(dev, perm):
    """[128, HB] image-half layout -> [IPIX] flat pixel data."""
    flat_k = np.ascontiguousarray(dev.T).reshape(-1)
    out = np.empty(IPIX, dtype=dev.dtype)
    out[perm] = flat_k
    return out


def kernel(logits, bboxes, labels):
    from concourse import bass_utils

    logits = np.asarray(logits, dtype=np.float32)
    bboxes = np.asarray(bboxes, dtype=np.int32)
    labels = np.asarray(labels, dtype=np.int32)

    gam, s0, num_rc = _host_gamma(bboxes)

    lab = labels.astype(np.float32)  # [B,H,W], IGNORE stays 255
    wmap = (labels != IGNORE).astype(np.float32) * gam
    enc = (2.0 * lab + wmap).astype(np.float16)  # [B,H,W]

    perms, ranges = _build_perms(labels)
    nc = _get_nc(ranges)

    lg16 = logits.astype(np.float16)  # [B,C,H,W]
    in_maps = []
    for i in range(N_CORES):
        img = [i * IPC, i * IPC + 1]
        # [C, 128, F]: image b's sorted pixels in columns [512b, 512b+512)
        lgi = np.concatenate(
            [
                _to_half(lg16[img[b]].reshape(C, IPIX), perms[img[b]])
                for b in range(2)
            ],
            axis=-1,
        )
        enci = np.concatenate(
            [_to_half(enc[img[b]].reshape(IPIX), perms[img[b]]) for b in range(2)],
            axis=-1,
        )
        in_maps.append({"logits16": lgi, "enc": enci})
    res = bass_utils.run_bass_kernel_spmd(nc, in_maps, core_ids=list(range(N_CORES)))

    logden = np.stack(
        [
            np.log(
                _from_half(
                    np.asarray(res.results[i // IPC]["den"])[
                        :, (i % IPC) * HB : (i % IPC + 1) * HB
                    ],
                    perms[i],
                ).astype(np.float32)
            ).reshape(H, W)
            for i in range(B)
        ],
        axis=0,
    )  # [B,H,W]
    loss_rc = _host_box_terms(logits, bboxes, logden)

    # weighted CE: sum w*logden (host, from exported map) - device gather sums
    wsum = (wmap.astype(np.float64) * logden.astype(np.float64)).reshape(B, -1).sum(axis=1)
    wce = 0.0
    for i in range(N_CORES):
        p = res.results[i]["partials"].astype(np.float64)
        for b in range(IPC):
            s1 = wsum[i * IPC + b] - p[:, b::2].sum()
            wce += s1 / s0[i * IPC + b]
    wce /= B

    out = LAMB * loss_rc / num_rc + wce
    return np.float32(out)


# revision 49
# speedup vs baseline: 1.0122x; 1.0008x over previous
"""Trainium2 Bass kernel for nn_Loss_PIP (PIP loss: box region terms + distance-map
weighted cross-entropy).

Strategy (data-parallel over batch across 8 NeuronCores, 2 images/core):
  - Pixel layout: the host deals each image's 65536 pixels into a
    [128 partitions x 512 column] half of the tile SORTED BY LABEL,
    round-robin across partitions (slot k -> partition k%128, column k//128;
    image b occupies columns [512b, 512b+512)). Every per-pixel computation
    (exp, denominator accumulation, den export, gather) is permutation-
    invariant, but the label-gather now only scans the narrow column range
    where class c of image b lives (~30 columns instead of 1024): the DVE
    gather drops from ~23.7us to ~4us and stops being the bottleneck. The
    class column ranges are computed from the actual labels at first call
    and compiled into the program (max across cores; stray neighbor-class
    pixels inside a range are killed by the enc gate).
  - Device (per core, SPMD): stream the 21 logit channel planes in fp16 (half
    the HBM bytes of f32); ACT computes exp (bf16, dual-channel ops to
    amortize fixed cost); PE accumulates the softmax denominator in PSUM via
    identity-matmul (identity built on-device on Pool); DVE runs one fused
    PIP_GATHER_DOT pass per (image, class) column range (driven by
    enc = 2*label + w) and the PSUM->SBUF den copies. The denominator is
    exported raw fp16; the host takes the log (no Ln => no activation-table
    switch). The last channel runs as two half planes to shorten the closing
    exp->matmul->copy->DMA chain; PSUM/staging tiles are split per bank to
    avoid false tile-granular WARs.
  - Host: Gamma weight-map pipeline (bbox-only), per-box window reductions on
    logden/logits, the w*logden reduction, permutation (un)packing, assembly.
"""

import sys

sys.path.insert(0, "/opt/trn_rl_repo")

import numpy as np

B, C, H, W = 16, 21, 256, 256
NB = 20
N_CORES = 8
IPC = B // N_CORES  # images per core
LAMB, ALPHA, TAU, R, SIGMA = 1.0, 0.5, 1.0, 3, 1.0
IGNORE = 255
F = 4 * W  # 1024 free elems per partition
HB = F // 2  # psum bank width in f32 = per-image column half
IPIX = H * W  # pixels per image = 128 * HB

_CACHE = {}


def _register_fused_op():
    """Register PIP_GATHER_DOT: out = m*(enc-s0)*in1, m = (enc-s0) in (s1, imm2);
    accum_out = sum(out). With enc = 2*label + w (w in {0} U (1,1.5)), s0=2c,
    s1=0.5, imm2=1.5 this computes w*(label==c)*logit in one DVE pass."""
    from concourse import dve_ops
    from concourse.dve_spec import C0, C1, C2, Spec, Src0, Src1, Zero, lower
    from concourse.dve_spec import _has_src1 as has_src1
    from concourse.dve_uop import DveOpSpec
    from operator import add as op_add
    import numpy as np_

    name = "PIP_GATHER_DOT"
    if name in dve_ops._SUB_OPCODE_FOR_NAME:
        return next(o for o in dve_ops.OPS if o.name == name)

    _t = Src0 - C0

    def _ref(in0, in1, s0, s1, imm2):
        t = in0.astype(np_.float32) - s0
        m = ((t > s1) & (t < imm2)).astype(np_.float32)
        b = (m * t * in1).astype(np_.float32)
        return b, b.reshape(b.shape[0], -1).sum(axis=-1, keepdims=True)

    spec = Spec(
        body=((_t > C1) & (_t < C2)) * _t * Src1,
        accum=op_add,
        accum_init=Zero,
        reference=_ref,
    )
    row = dve_ops._CUSTOM_DVE_ROW_BASE + len(dve_ops.OPS)
    assert row < 0x20
    shas = {}
    for ver in ("v3", "v4"):
        try:
            uops = lower(spec, ver=ver)
        except Exception:
            continue
        shas[ver] = DveOpSpec(
            name=name, opcode=row, uops=uops, rd1_en=has_src1(spec)
        ).sha(ver)
    op = dve_ops.DveOp(name, spec, subdim=False, uops_sha=shas)
    dve_ops.OPS.append(op)
    dve_ops.CUSTOM_DVE_SPECS[name] = spec
    dve_ops._SUB_OPCODE_FOR_NAME[name] = row
    return op


def _build_nc(ranges):
    """ranges[b][c] = (start, end) absolute column range (image b's half)
    covering every partition's class-c pixels across all cores."""
    import concourse.bacc as bacc
    import concourse.mybir as mybir
    from concourse import tile

    dt = mybir.dt
    Alu = mybir.AluOpType
    Act = mybir.ActivationFunctionType

    nc = bacc.Bacc(
        "TRN2",
        target_bir_lowering=False,
        debug=False,
        enable_asserts=False,
        num_devices=N_CORES,
    )

    # host supplies logits label-sorted + fp16: [c, p, f] = logits[pix(p,f)]
    logits16 = nc.dram_tensor("logits16", [C, 128, F], dt.float16, kind="ExternalInput")
    enc_in = nc.dram_tensor("enc", [128, F], dt.float16, kind="ExternalInput")
    den_out = nc.dram_tensor("den", [128, F], dt.float16, kind="ExternalOutput")
    # parts col 2c+b = sum_p w*(label==c)*logit_c over image b
    partials_out = nc.dram_tensor(
        "partials", [128, 2 * C], dt.float32, kind="ExternalOutput"
    )

    fused = _register_fused_op()

    with tile.TileContext(nc) as tc:
        with (
            tc.tile_pool(name="persist", bufs=1) as pp,
            tc.tile_pool(name="stream", bufs=4) as sp,
            tc.tile_pool(name="psum", bufs=1, space="PSUM") as psp,
        ):
            enc = pp.tile([128, F], dt.float16, name="enc")
            ident = pp.tile([128, 128], dt.bfloat16, name="ident")
            ones = pp.tile([128, 128], dt.bfloat16, name="ones")
            parts = pp.tile([128, 2 * C], dt.float32, name="parts")
            # separate PSUM/SBUF tiles per bank half: no false WAR between the
            # bank-0 epilogue and bank-1 accumulation
            dps = [psp.tile([128, HB], dt.float32, name=f"dps{h}") for h in range(2)]
            dsb = [pp.tile([128, HB], dt.float16, name=f"dsb{h}") for h in range(2)]

            # identity for the PE accumulate, generated on the Pool engine
            nc.gpsimd.memset(ones[:, :], 1.0)
            nc.gpsimd.affine_select(
                out=ident[:, :],
                in_=ones[:, :],
                pattern=[[1, 128]],
                compare_op=Alu.is_equal,
                fill=0.0,
                base=0,
                channel_multiplier=-1,
            )

            # ---- input stream on the SP queue ----------------------------
            lg = {}
            lg_dual = {}

            def dma_lg(c):
                t = sp.tile([128, F], dt.float16, name=f"lg{c}", tag="lg", bufs=C)
                nc.sync.dma_start(out=t[:, :], in_=logits16[c])
                lg[c] = t

            def dma_lgn(c, n):  # channels c..c+n-1 in one DMA / one tile
                t = sp.tile(
                    [128, n * F], dt.float16, name=f"lg{c}", tag=f"lg{n}x", bufs=4
                )
                nc.sync.dma_start(
                    out=t[:, :].rearrange("p (c f) -> p c f", c=n),
                    in_=logits16[c : c + n].rearrange("c p f -> p c f"),
                )
                lg_dual[c] = t
                for k in range(n):
                    lg[c + k] = t[:, k * F : (k + 1) * F]

            # exp groups grow as the pipeline warms (amortizing ACT's 185ns
            # per-op cost), then shrink again near the tail so the PE matmuls
            # of the last groups don't bunch up and delay the stop matmuls
            GROUPS = [(0,), (1,), (2, 3), (4, 5), (6, 7, 8), (9, 10, 11),
                      (12, 13, 14, 15), (16, 17), (18, 19)]
            # enc rides late: the DVE gathers are tiny and have slack, while
            # ACT is arrival-paced at the head — the early slots go to lg
            for g in GROUPS[:7]:
                dma_lgn(g[0], len(g))
            nc.sync.dma_start(out=enc[:, :], in_=enc_in[:, :])
            for g in GROUPS[7:]:
                dma_lgn(g[0], len(g))
            # last channel in two half tiles so the tail chain is one half
            lg20 = [
                sp.tile([128, HB], dt.float16, name=f"lg20{h}", tag="lg20", bufs=2)
                for h in range(2)
            ]
            for h in range(2):
                nc.sync.dma_start(
                    out=lg20[h][:, :], in_=logits16[20][:, h * HB : (h + 1) * HB]
                )

            # ---- per-channel compute -------------------------------------
            wmax = max(e - s for rb in ranges for (s, e) in rb)

            def gather_dve(c, in1_tile, col0):
                # one pass per image half over class c's column range
                for b in range(2):
                    s, e = ranges[b][c]
                    tout = sp.tile(
                        [128, wmax], dt.float16, name="tout", tag="tout", bufs=2
                    )
                    nc.vector._custom_dve(
                        fused,
                        out=tout[:, : e - s],
                        in0=enc[:, s:e],
                        in1=in1_tile[:, s - col0 : e - col0],
                        s0=2.0 * c,
                        s1=0.5,
                        imm2=1.5,
                        accum_out=parts[:, 2 * c + b : 2 * c + b + 1],
                    )

            last_bank1 = []  # (ex-slice) of the final group, deferred

            for gi, g in enumerate(GROUPS):
                n = len(g)
                last = gi == len(GROUPS) - 1
                exg = sp.tile(
                    [128, n * F], dt.bfloat16, name="ex", tag=f"ex{n}x", bufs=3
                )
                nc.scalar.activation(
                    out=exg[:, :], in_=lg_dual[g[0]][:, :], func=Act.Exp
                )
                for k, cc in enumerate(g):
                    exk = exg[:, k * F : (k + 1) * F]
                    for h in range(2):
                        if last and h == 1:
                            # defer the final group's bank1 matmuls past the
                            # bank0 stop so mm_a (and the den_a path) isn't
                            # stuck behind this group's matmul bunch
                            last_bank1.append(exk)
                            continue
                        nc.tensor.matmul(
                            dps[h][:, :],
                            ident[:, :],
                            exk[:, h * HB : (h + 1) * HB],
                            start=(cc == 0),
                            stop=False,
                        )
                    gather_dve(cc, lg[cc], 0)
            # c20: per-half exp -> stop matmul; image b's class-20 range sits
            # inside half-tile b (class 20 sorts just before the IGNORE block)
            assert ranges[0][20][1] <= HB and ranges[1][20][0] >= HB, ranges
            exh = []
            for h in range(2):
                e = sp.tile([128, HB], dt.bfloat16, name="exh", tag="exh", bufs=2)
                nc.scalar.activation(out=e[:, :], in_=lg20[h][:, :], func=Act.Exp)
                exh.append(e)
            nc.tensor.matmul(
                dps[0][:, :], ident[:, :], exh[0][:, :], start=False, stop=True
            )
            for exk in last_bank1:
                nc.tensor.matmul(
                    dps[1][:, :], ident[:, :], exk[:, HB:F], start=False, stop=False
                )
            nc.tensor.matmul(
                dps[1][:, :], ident[:, :], exh[1][:, :], start=False, stop=True
            )
            for b in range(2):
                s, e = ranges[b][20]
                tout = sp.tile(
                    [128, wmax], dt.float16, name="tout", tag="tout", bufs=2
                )
                nc.vector._custom_dve(
                    fused,
                    out=tout[:, : e - s],
                    in0=enc[:, s:e],
                    in1=lg20[b][:, s - b * HB : e - b * HB],
                    s0=2.0 * 20,
                    s1=0.5,
                    imm2=1.5,
                    accum_out=parts[:, 40 + b : 41 + b],
                )
            # partials out on SP before the den DMAs (it is ready earlier)
            nc.sync.dma_start(out=partials_out[:, :], in_=parts[:, :])
            # both den copies on the idle DVE: copy_a right after mm_a (not
            # stuck behind ACT's remaining exps), so den_a's HWDGE descriptor
            # gen clears the shared HWDGE stage before den_b's gen needs it;
            # copy_b still starts at mm_b+sem either way. den_a DMA issues
            # from the (exp-free) ACT queue, den_b from SP.
            nc.vector.tensor_copy(out=dsb[0][:, :], in_=dps[0][:, :])
            nc.scalar.dma_start(out=den_out[:, 0:HB], in_=dsb[0][:, :])
            nc.scalar.activation(out=dsb[1][:, :], in_=dps[1][:, :], func=Act.Copy)
            nc.sync.dma_start(out=den_out[:, HB:F], in_=dsb[1][:, :])

    nc.compile()
    return nc


def _get_nc(ranges=None):
    if ranges is None:
        # introspection path (test harness): return the program built by the
        # latest kernel() call
        return next(reversed(_CACHE.values()))
    key = tuple(tuple(r) for r in (ranges[0] + ranges[1]))
    if key not in _CACHE:
        _CACHE[key] = _build_nc(ranges)
    return _CACHE[key]


def _gauss_1d():
    x = np.arange(2 * R + 1, dtype=np.float64) - R
    g = np.exp(-(x**2) / (2.0 * SIGMA**2))
    return (g / g.sum()).astype(np.float32)


def _host_gamma(bboxes):
    """Gamma weight maps [B,H,W] plus per-image Gamma sums; depends only on bboxes."""
    bb = bboxes.reshape(B * NB, 5).astype(np.int64)
    x0, y0, x1, y1, cls = bb[:, 0], bb[:, 1], bb[:, 2], bb[:, 3], bb[:, 4]
    valid = cls != -1
    ys = np.arange(H)
    xs = np.arange(W)
    row_m = (ys[None, :] >= y0[:, None]) & (ys[None, :] <= y1[:, None])  # [M,H]
    col_m = (xs[None, :] >= x0[:, None]) & (xs[None, :] <= x1[:, None])  # [M,W]
    in_r = (ys[None, :] > y0[:, None]) & (ys[None, :] < y1[:, None])
    in_c = (xs[None, :] > x0[:, None]) & (xs[None, :] < x1[:, None])

    nop = np.ones((B, H, W), dtype=np.float32)
    dis = np.zeros((B, H, W), dtype=np.float32)
    for m in range(B * NB):
        if not valid[m]:
            continue
        b = m // NB
        full = np.outer(row_m[m], col_m[m]).astype(np.float32)
        inner = np.outer(in_r[m], in_c[m]).astype(np.float32)
        nop[b] += full
        dis[b] += full * (1.0 - inner)

    g = _gauss_1d().astype(np.float64)
    # reflect-pad + separable 7x7 gaussian (matches conv with outer(g, g), 'VALID')
    disp = np.pad(dis, ((0, 0), (R, R), (0, 0)), mode="reflect").astype(np.float64)
    tmp = np.zeros((B, H, W), dtype=np.float64)
    for k in range(2 * R + 1):
        tmp += g[k] * disp[:, k : k + H, :]
    tmp = np.pad(tmp, ((0, 0), (0, 0), (R, R)), mode="reflect")
    blur = np.zeros((B, H, W), dtype=np.float64)
    for k in range(2 * R + 1):
        blur += g[k] * tmp[:, :, k : k + W]
    dis_b = blur.astype(np.float32) + 1.0

    nd = nop * dis_b
    ndmax = nd.max()
    sig = 1.0 / (1.0 + np.exp(-(nd / ndmax).astype(np.float64)))
    gam = ((sig - 0.5) * TAU + 1.0).astype(np.float32)
    s0 = gam.reshape(B, -1).astype(np.float64).sum(axis=1)  # per-image Gamma sums

    h = y1 - y0 + 1
    w = x1 - x0 + 1
    num_rc = 1e-5 + float(np.where(valid, h + w, 0).sum())
    return gam, s0, num_rc


def _host_box_terms(logits, bboxes, logden):
    """loss_rc from per-box window reductions on log-prob maps."""
    bb = bboxes.reshape(B * NB, 5).astype(np.int64)
    term = 0.0
    for m in range(B * NB):
        x0, y0, x1, y1, cls = bb[m]
        if cls == -1:
            continue
        b = m // NB
        lp = (
            logits[b, cls, y0 : y1 + 1, x0 : x1 + 1].astype(np.float64)
            - logden[b, y0 : y1 + 1, x0 : x1 + 1].astype(np.float64)
        )
        colmax = lp.max(axis=0)
        rowmax = lp.max(axis=1)
        colmin = lp.min(axis=0)
        rowmin = lp.min(axis=1)
        term += ALPHA * (colmax.sum() + rowmax.sum())
        term += (1.0 - ALPHA) * (
            np.log1p(-np.exp(colmin)).sum() + np.log1p(-np.exp(rowmin)).sum()
        )
    return -term


def _build_perms(labels):
    """Per-image label-sorted pixel permutations + per-(image-slot, class)
    absolute column ranges shared across cores. Within image b's column half,
    slot k (k = (col-512b)*128 + partition) holds pixel perm[k] of the
    image's flat [H*W] pixel index space."""
    perms = np.empty((B, IPIX), dtype=np.int64)
    Ks = np.empty((B, C + 2), dtype=np.int64)
    for i in range(B):
        lab = labels[i].reshape(-1)
        perms[i] = np.argsort(lab, kind="stable")
        cnt = np.bincount(np.minimum(lab, C), minlength=C + 1)
        Ks[i] = np.concatenate([[0], np.cumsum(cnt)])
    ranges = [[], []]
    for b in range(2):
        imgs = [i * IPC + b for i in range(N_CORES)]
        for c in range(C):
            s = min(int(Ks[i][c]) // 128 for i in imgs)
            e = max((int(Ks[i][c + 1]) - 1) // 128 + 1 for i in imgs)
            ranges[b].append((b * HB + s, b * HB + e))
    return perms, ranges


def _to_half(flat, perm):
    """flat [..., IPIX] pixel data -> [..., 128, HB] image-half layout."""
    a = np.take(flat, perm, axis=-1)
    a = a.reshape(*a.shape[:-1], HB, 128)
    return np.ascontiguousarray(np.swapaxes(a, -1, -2))


def _from_half(dev, perm):
    """[128, HB] image-half layout -> [IPIX] flat pixel data."""
    flat_k = np.ascontiguousarray(dev.T).reshape(-1)
    out = np.empty(IPIX, dtype=dev.dtype)
    out[perm] = flat_k
    return out


def kernel(logits, bboxes, labels):
    from concourse import bass_utils

    logits = np.asarray(logits, dtype=np.float32)
    bboxes = np.asarray(bboxes, dtype=np.int32)
    labels = np.asarray(labels, dtype=np.int32)

    gam, s0, num_rc = _host_gamma(bboxes)

    lab = labels.astype(np.float32)  # [B,H,W], IGNORE stays 255
    wmap = (labels != IGNORE).astype(np.float32) * gam
    enc = (2.0 * lab + wmap).astype(np.float16)  # [B,H,W]

    perms, ranges = _build_perms(labels)
    nc = _get_nc(ranges)

    lg16 = logits.astype(np.float16)  # [B,C,H,W]
    in_maps = []
    for i in range(N_CORES):
        img = [i * IPC, i * IPC + 1]
        # [C, 128, F]: image b's sorted pixels in columns [512b, 512b+512)
        lgi = np.concatenate(
            [
                _to_half(lg16[img[b]].reshape(C, IPIX), perms[img[b]])
                for b in range(2)
            ],
            axis=-1,
        )
        enci = np.concatenate(
            [_to_half(enc[img[b]].reshape(IPIX), perms[img[b]]) for b in range(2)],
            axis=-1,
        )
        in_maps.append({"logits16": lgi, "enc": enci})
    res = bass_utils.run_bass_kernel_spmd(nc, in_maps, core_ids=list(range(N_CORES)))

    logden = np.stack(
        [
            np.log(
                _from_half(
                    np.asarray(res.results[i // IPC]["den"])[
                        :, (i % IPC) * HB : (i % IPC + 1) * HB
                    ],
                    perms[i],
                ).astype(np.float32)
            ).reshape(H, W)
            for i in range(B)
        ],
        axis=0,
    )  # [B,H,W]
    loss_rc = _host_box_terms(logits, bboxes, logden)

    # weighted CE: sum w*logden (host, from exported map) - device gather sums
    wsum = (wmap.astype(np.float64) * logden.astype(np.float64)).reshape(B, -1).sum(axis=1)
    wce = 0.0
    for i in range(N_CORES):
        p = res.results[i]["partials"].astype(np.float64)
        for b in range(IPC):
            s1 = wsum[i * IPC + b] - p[:, b::2].sum()
            wce += s1 / s0[i * IPC + b]
    wce /= B

    out = LAMB * loss_rc / num_rc + wce
    return np.float32(out)
